# revision 12
# baseline (speedup 1.0000x reference)
"""BiLSTM-CRF NLL kernel for Trainium2, 8 NeuronCores, data-parallel over batch.

Per core: 4 sequences through embedding gather -> 3 BiLSTM layers -> projection
-> CRF forward algorithm + gold score. Output: per-core partial of
(log_Z - score)/B; host sums the 8 partials.

Layouts (per core):
  - hT state/output buffers: [128, (k, slot, b)] bf16, hidden-dim on partitions.
    fw h_t at slot t+1 (k 0..1), bw h_t at slot t (k 2..3); slot axis size S+1.
  - gates: custom gate order [i, f, o, g] so sigmoid gates are contiguous.
  - CRF state e = exp(la - corr): [128, 1] bf16, partition = 32*b + tag.
    One resident 128x128 block-diag exp(transitions^T) matmul per step.
"""
import numpy as np
import ml_dtypes

import concourse.bacc as bacc
import concourse.bass as bass
import concourse.mybir as mybir
import concourse.tile as tile
from concourse.bass import ds
from concourse.bass_utils import run_bass_kernel_spmd
from concourse.masks import make_identity

BF = mybir.dt.bfloat16
F32 = mybir.dt.float32
I32 = mybir.dt.int32
AF = mybir.ActivationFunctionType
OP = mybir.AluOpType

VOCAB, EMB, HID, TAGS, LAYERS, B, S_FULL = 50000, 256, 512, 32, 3, 32, 256
H = HID // 2  # 256, per direction
BL = 4        # batch per core
NCORES = 8
NEG = -1e30

# permuted gate-chunk order: [i, f, o, g] (original packed order i,f,g,o)
PERM_CHUNKS = [0, 1, 2, 3, 6, 7, 4, 5]  # orig 128-row chunk ids in new order


def _perm_rows(w):
    """Reorder gate rows of [4H, ...] from (i,f,g,o) to (i,f,o,g)."""
    chunks = w.reshape(8, 128, *w.shape[1:])
    return np.concatenate([chunks[c] for c in PERM_CHUNKS], axis=0)


def build_nc(S=S_FULL, unroll=8, crf_k=12, debug_outs=True):
    SLOT = S + 1
    nc = bacc.Bacc("TRN2", target_bir_lowering=False, debug=False)
    NT = (S * BL) // 128          # token tiles (8 at S=256)
    NTOK = S * BL

    # ---------------- DRAM tensors ----------------
    emb = nc.dram_tensor("emb", [VOCAB, EMB], BF, kind="ExternalInput")
    xidx = nc.dram_tensor("xidx", [NTOK, 1], I32, kind="ExternalInput")
    wih = [[nc.dram_tensor(f"wih{l}{d}", [128, (2 if l == 0 else 4) * 8 * 128], BF,
                           kind="ExternalInput") for d in range(2)] for l in range(LAYERS)]
    whh = [[nc.dram_tensor(f"whh{l}{d}", [128, 16 * 128], BF, kind="ExternalInput")
            for d in range(2)] for l in range(LAYERS)]
    bias = [[nc.dram_tensor(f"bias{l}{d}", [128, 8], F32, kind="ExternalInput")
             for d in range(2)] for l in range(LAYERS)]
    woutT = nc.dram_tensor("woutT", [128, 4 * TAGS], BF, kind="ExternalInput")
    bout = nc.dram_tensor("bout", [1, TAGS], BF, kind="ExternalInput")
    blockraw = nc.dram_tensor("blockraw", [128, 128], F32, kind="ExternalInput")
    startrep = nc.dram_tensor("startrep", [128, 1], F32, kind="ExternalInput")
    endrep = nc.dram_tensor("endrep", [128, 1], F32, kind="ExternalInput")
    p4 = nc.dram_tensor("p4", [128, BL], F32, kind="ExternalInput")
    emone = nc.dram_tensor("emone", [128, NT * TAGS], F32, kind="ExternalInput")
    pf = nc.dram_tensor("pf", [TAGS, NTOK + 128], F32, kind="ExternalInput")
    pt = nc.dram_tensor("pt", [128, (NT + 1) * 34], F32, kind="ExternalInput")
    tables = nc.dram_tensor("tables", [TAGS, 34], F32, kind="ExternalInput")
    out = nc.dram_tensor("out", [1, 1], F32, kind="ExternalOutput")
    if debug_outs:
        dbg_em = nc.dram_tensor("dbg_em", [128, NT * TAGS], F32, kind="ExternalOutput")
        dbg_h = nc.dram_tensor("dbg_h", [128, 4 * SLOT * BL], F32, kind="ExternalOutput")

    with tile.TileContext(nc) as tc:
        with tc.tile_pool(name="const", bufs=1) as cpool, \
             tc.tile_pool(name="big", bufs=1) as bigpool, \
             tc.tile_pool(name="wihp", bufs=2) as wihpool, \
             tc.tile_pool(name="gath", bufs=3) as gathpool, \
             tc.tile_pool(name="tmp", bufs=6) as tmppool, \
             tc.tile_pool(name="actp", bufs=4) as actpool:

            def C(shape, dtype, tag):
                return cpool.tile(shape, dtype, tag=tag, name=tag)

            # ---------- constants to SBUF ----------
            id128b = C([128, 128], BF, "id128b")
            make_identity(nc, id128b[:])
            id128f = C([128, 128], F32, "id128f")
            make_identity(nc, id128f[:])
            id1b = C([1, 1], BF, "id1b")
            nc.gpsimd.memset(id1b[:], 1.0)
            ones128 = C([128, 1], F32, "ones128")
            nc.gpsimd.memset(ones128[:], 1.0)
            onesrow_b = C([1, 128], BF, "onesrowb")
            nc.gpsimd.memset(onesrow_b[:], 1.0)

            whh_t, bias_t = {}, {}
            for l in range(LAYERS):
                for d in range(2):
                    wt = C([128, 16 * 128], BF, f"whht{l}{d}")
                    nc.sync.dma_start(wt[:], whh[l][d].ap())
                    whh_t[l, d] = wt
                    bt = C([128, 8], F32, f"biast{l}{d}")
                    nc.sync.dma_start(bt[:], bias[l][d].ap())
                    bias_t[l, d] = bt

            woutT_t = C([128, 4 * TAGS], BF, "woutTt")
            nc.sync.dma_start(woutT_t[:], woutT.ap())
            bout_t = C([1, TAGS], BF, "boutt")
            nc.sync.dma_start(bout_t[:], bout.ap())
            blockraw_t = C([128, 128], F32, "blockrawt")
            nc.sync.dma_start(blockraw_t[:], blockraw.ap())
            startrep_t = C([128, 1], F32, "startrept")
            nc.sync.dma_start(startrep_t[:], startrep.ap())
            endrep_t = C([128, 1], F32, "endrept")
            nc.sync.dma_start(endrep_t[:], endrep.ap())
            p4_t = C([128, BL], F32, "p4t")
            nc.sync.dma_start(p4_t[:], p4.ap())
            emone_t = C([128, NT * TAGS], F32, "emonet")
            nc.sync.dma_start(emone_t[:], emone.ap())
            pf_t = C([TAGS, NTOK + 128], F32, "pft")
            nc.sync.dma_start(pf_t[:], pf.ap())
            pt_t = C([128, (NT + 1) * 34], F32, "ptt")
            nc.sync.dma_start(pt_t[:], pt.ap())
            tables_t = C([TAGS, 34], F32, "tablest")
            nc.sync.dma_start(tables_t[:], tables.ap())
            idx_t = C([128, NT], I32, "idxt")
            nc.sync.dma_start(
                idx_t[:], xidx.ap().rearrange("(n p) o -> p (n o)", p=128))

            blockexp = C([128, 128], BF, "blockexp")
            nc.scalar.activation(blockexp[:], blockraw_t[:], AF.Exp)
            expstart = C([128, 1], F32, "expstart")
            nc.scalar.activation(expstart[:], startrep_t[:], AF.Exp)
            expend = C([128, 1], F32, "expend")
            nc.scalar.activation(expend[:], endrep_t[:], AF.Exp)

            # ---------- embedding gather -> embT [128, (k2)(t)(b)] bf16 ----------
            embT = bigpool.tile([128, 2 * S * BL], BF, tag="embT", name="embT")
            psemb = tc.alloc_tile_pool(name="psemb", bufs=2, space="PSUM")
            for n in range(NT):
                g = gathpool.tile([128, EMB], BF, tag="gather")
                nc.gpsimd.indirect_dma_start(
                    out=g[:], out_offset=None, in_=emb.ap(),
                    in_offset=bass.IndirectOffsetOnAxis(ap=idx_t[:, n:n + 1], axis=0))
                pT = psemb.tile([128, 256], BF, tag="embtr")
                for k in range(2):
                    nc.tensor.transpose(pT[:, k * 128:(k + 1) * 128],
                                        g[:, k * 128:(k + 1) * 128], id128b[:])
                for k in range(2):
                    nc.scalar.copy(
                        embT[:, k * S * BL + n * 128: k * S * BL + (n + 1) * 128],
                        pT[:, k * 128:(k + 1) * 128])

            psemb.release()

            # ---------- layers ----------
            h_all = [bigpool.tile([128, 4 * SLOT * BL], BF, tag=f"hall{l}",
                                  name=f"hall{l}") for l in range(LAYERS)]
            xpart = [bigpool.tile([128, 8 * S * BL], F32, tag=f"xpart{d}",
                                  name=f"xpart{d}") for d in range(2)]

            for l in range(LAYERS):
                n_k = 2 if l == 0 else 4
                wih_t = wihpool.tile([128, 2 * 4 * 8 * 128], BF, tag="wih")
                w5 = wih_t.rearrange("p (d k m q) -> p d k m q", d=2, k=4, m=8)
                for d in range(2):
                    nc.sync.dma_start(
                        wih_t[:, d * 4096: d * 4096 + n_k * 8 * 128], wih[l][d].ap())

                if l == 0:
                    def src_ap(k, lo, sz):
                        return embT[:, k * (S * BL) + lo: k * (S * BL) + lo + sz]
                else:
                    hp = h_all[l - 1]
                    def src_ap(k, lo, sz, hp=hp):
                        off = k * (SLOT * BL) + (BL if k < 2 else 0)
                        return hp[:, off + lo: off + lo + sz]

                CH = min(512, S * BL)
                NCH = (S * BL) // CH
                ps512 = tc.alloc_tile_pool(name=f"psx{l}", bufs=4, space="PSUM")
                for d in range(2):
                    xp = xpart[d]
                    for m in range(8):
                        pchunks = [ps512.tile([128, CH], F32, tag="xpps", name="xpps")
                                   for _ in range(NCH)]
                        for k in range(n_k):
                            for c in range(NCH):
                                nc.tensor.matmul(
                                    pchunks[c][:], w5[:, d, k, m], src_ap(k, c * CH, CH),
                                    start=(k == 0), stop=(k == n_k - 1))
                        for c in range(NCH):
                            dst = xp[:, m * S * BL + c * CH: m * S * BL + c * CH + CH]
                            if c % 2 == 0:
                                nc.scalar.activation(dst, pchunks[c][:], AF.Identity,
                                                     bias=bias_t[l, d][:, m:m + 1])
                            else:
                                nc.vector.tensor_scalar_add(
                                    dst, pchunks[c][:], bias_t[l, d][:, m:m + 1])

                ps512.release()

                # ---------- recurrent scan ----------
                psg = tc.alloc_tile_pool(name=f"psg{l}", bufs=4, space="PSUM")
                ha = h_all[l]
                nc.vector.memset(ha[:, 0:BL], 0.0)
                nc.vector.memset(ha[:, SLOT * BL: SLOT * BL + BL], 0.0)
                nc.vector.memset(ha[:, 2 * SLOT * BL + S * BL: 2 * SLOT * BL + SLOT * BL], 0.0)
                nc.vector.memset(ha[:, 3 * SLOT * BL + S * BL: 3 * SLOT * BL + SLOT * BL], 0.0)
                c_st = [[C([128, 2 * BL], F32, f"cst{l}{d}{j}") for j in range(2)]
                        for d in range(2)]
                for d in range(2):
                    nc.vector.memset(c_st[d][1][:], 0.0)  # cprev for t=0 (parity (0+1)%2)
                xpd = [xpart[d].rearrange("p (m tb) -> p m tb", m=8) for d in range(2)]

                assert S % unroll == 0
                with tc.For_i(0, S // unroll, 1) as iv:
                    for u in range(unroll):
                        t_fw = iv * unroll + u
                        for d in range(2):
                            if d == 0:
                                t_x = t_fw
                                rd_slot = t_fw            # h_{t-1} at slot t
                                wr_slot = t_fw + 1
                            else:
                                t_x = (S - 1) - t_fw      # bw time
                                rd_slot = (S - t_fw)      # h_{tau+1} at slot tau+1
                                wr_slot = (S - 1) - t_fw
                            kbase = 2 * d
                            pg = psg.tile([128, 32], F32, tag="gates")
                            for m in range(8):
                                for k in range(2):
                                    nc.tensor.matmul(
                                        pg[:, m * 4:(m + 1) * 4],
                                        whh_t[l, d][:, (m * 2 + k) * 128:(m * 2 + k + 1) * 128],
                                        ha[:, ds((kbase + k) * (SLOT * BL) + rd_slot * BL, BL)],
                                        start=(k == 0), stop=(k == 1))
                            pre = tmppool.tile([128, 32], F32, tag="pre")
                            nc.vector.tensor_tensor(
                                out=pre[:], in0=pg[:],
                                in1=xpd[d][:, :, ds(t_x * BL, BL)], op=OP.add)
                            av = actpool.tile([128, 32], F32, tag="act")
                            nc.scalar.activation(av[:, 0:24], pre[:, 0:24], AF.Sigmoid)
                            nc.scalar.activation(av[:, 24:32], pre[:, 24:32], AF.Tanh)
                            ig = tmppool.tile([128, 8], F32, tag="ig")
                            nc.vector.tensor_tensor(out=ig[:], in0=av[:, 0:8],
                                                    in1=av[:, 24:32], op=OP.mult)
                            cprev = c_st[d][(u + 1) % 2]
                            cnew = c_st[d][u % 2]
                            fc = tmppool.tile([128, 8], F32, tag="fc")
                            nc.vector.tensor_tensor(out=fc[:], in0=av[:, 8:16],
                                                    in1=cprev[:], op=OP.mult)
                            nc.vector.tensor_tensor(out=cnew[:], in0=ig[:], in1=fc[:],
                                                    op=OP.add)
                            tct = tmppool.tile([128, 8], F32, tag="tct")
                            nc.scalar.activation(tct[:], cnew[:], AF.Tanh)
                            hw = ha.rearrange("p (k s b) -> p k s b", k=4, s=SLOT)
                            nc.vector.tensor_tensor(
                                out=hw[:, kbase:kbase + 2, ds(wr_slot, 1), :],
                                in0=av[:, 16:24], in1=tct[:], op=OP.mult)

                psg.release()

            # ---------- projection / emissions ----------
            pspr = tc.alloc_tile_pool(name="pspr", bufs=2, space="PSUM")
            em_row = bigpool.tile([128, NT * TAGS], F32, tag="emrow", name="emrow")
            expem_col = bigpool.tile([128, S], F32, tag="expemcol", name="expemcol")
            negmax = C([128, NT], F32, "negmax")
            h3 = h_all[LAYERS - 1]
            for n in range(NT):
                pe = pspr.tile([128, TAGS], F32, tag="emps")
                for k in range(4):
                    off = k * (SLOT * BL) + (BL if k < 2 else 0) + n * 128
                    nc.tensor.matmul(pe[:], h3[:, off:off + 128],
                                     woutT_t[:, k * TAGS:(k + 1) * TAGS],
                                     start=(k == 0), stop=False)
                nc.tensor.matmul(pe[:], onesrow_b[:], bout_t[:], start=False, stop=True)
                nc.vector.tensor_reduce(out=negmax[:, n:n + 1], in_=pe[:],
                                        axis=mybir.AxisListType.X, op=OP.max, negate=True)
                nc.scalar.activation(em_row[:, n * TAGS:(n + 1) * TAGS], pe[:], AF.Copy)
                exr = tmppool.tile([128, TAGS], F32, tag="exr")
                nc.scalar.activation(exr[:], pe[:], AF.Exp, bias=negmax[:, n:n + 1])
                pT = pspr.tile([TAGS, 128], F32, tag="emtr")
                nc.tensor.transpose(pT[:], exr[:], id128f[:])
                pT3 = pT.rearrange("j (t b) -> j t b", b=BL)
                for b in range(BL):
                    nc.scalar.copy(
                        expem_col[32 * b:32 * b + 32, n * 32:(n + 1) * 32],
                        pT3[:, :, b])

            # C_row init = sum_t max_j(em) per b  (negmax holds -max)
            rowsum = tmppool.tile([128, 1], F32, tag="rowsum")
            nc.vector.tensor_reduce(out=rowsum[:], in_=negmax[:],
                                    axis=mybir.AxisListType.X, op=OP.add)
            pc = pspr.tile([1, BL], F32, tag="cinit", bufs=1)
            nc.tensor.matmul(pc[:], rowsum[:], p4_t[:], start=True, stop=True)
            c_row = C([1, BL], F32, "crow")
            nc.scalar.mul(c_row[:], pc[:], -1.0)

            pspr.release()

            # ---------- gold score ----------
            pssc = tc.alloc_tile_pool(name="pssc", bufs=2, space="PSUM")
            scoreacc = C([128, 2 * NT + 1], F32, "scoreacc")
            pfr = pf_t.rearrange("j (n p) -> j n p", p=128)
            ptr = pt_t.rearrange("p (n c) -> p n c", c=34)
            for n in range(NT + 1):
                psc = pssc.tile([128, 34], F32, tag="scps")
                nc.tensor.matmul(psc[:], pfr[:, n], tables_t[:], start=True, stop=True)
                sct = tmppool.tile([128, 34], F32, tag="sctmp")
                nc.vector.scalar_tensor_tensor(
                    out=sct[:], in0=psc[:], scalar=1.0, in1=ptr[:, n],
                    op0=OP.mult, op1=OP.mult, accum_out=scoreacc[:, n:n + 1])
            emr = em_row.rearrange("p (n j) -> p n j", j=TAGS)
            emo = emone_t.rearrange("p (n j) -> p n j", j=TAGS)
            for n in range(NT):
                sct = tmppool.tile([128, TAGS], F32, tag="sctmp")
                nc.vector.scalar_tensor_tensor(
                    out=sct[:], in0=emr[:, n], scalar=1.0, in1=emo[:, n],
                    op0=OP.mult, op1=OP.mult,
                    accum_out=scoreacc[:, NT + 1 + n:NT + 2 + n])
            pscore = pssc.tile([1, 2 * NT + 1], F32, tag="scsum", bufs=1)
            nc.tensor.matmul(pscore[:], ones128[:], scoreacc[:], start=True, stop=True)
            score_tot = tmppool.tile([1, 1], F32, tag="scot")
            nc.vector.tensor_reduce(out=score_tot[:], in_=pscore[:],
                                    axis=mybir.AxisListType.X, op=OP.add)

            pssc.release()

            # ---------- CRF forward scan ----------
            pscrf = tc.alloc_tile_pool(name="pscrf", bufs=2, space="PSUM")
            e_a = C([128, 1], BF, "ea")
            e_b = C([128, 1], BF, "eb")
            einit = tmppool.tile([128, 1], F32, tag="einit")
            nc.vector.tensor_tensor(out=einit[:], in0=expstart[:],
                                    in1=expem_col[:, 0:1], op=OP.mult)
            nc.vector.tensor_copy(e_a[:], einit[:])

            n_main = (S - 1) // crf_k
            tail = (S - 1) - n_main * crf_k
            assert crf_k % 2 == 0

            def crf_step(cur, nxt, col_expr):
                pe1 = pscrf.tile([128, 1], F32, tag="crfps")
                nc.tensor.matmul(pe1[:], blockexp[:], cur[:], start=True, stop=True)
                nc.vector.tensor_tensor(out=nxt[:], in0=pe1[:],
                                        in1=expem_col[:, col_expr], op=OP.mult)

            def rescale(cur):
                prow = pscrf.tile([1, 128], BF, tag="erow", bufs=1)
                nc.tensor.transpose(prow[:], cur[:], id128b[:])
                erow = tmppool.tile([1, 128], F32, tag="erowf")
                nc.scalar.copy(erow[:], prow[:])
                mx = tmppool.tile([1, BL], F32, tag="emx")
                nc.vector.tensor_reduce(
                    out=mx[:], in_=erow.rearrange("o (b j) -> o b j", b=BL),
                    axis=mybir.AxisListType.X, op=OP.max)
                rc = tmppool.tile([1, BL], F32, tag="erc")
                nc.vector.reciprocal(rc[:], mx[:])
                esc = tmppool.tile([1, 128], BF, tag="escl")
                nc.vector.tensor_tensor(
                    out=esc.rearrange("o (b j) -> o b j", b=BL),
                    in0=erow.rearrange("o (b j) -> o b j", b=BL),
                    in1=rc[:].to_broadcast([1, BL, 32]), op=OP.mult)
                pcol = pscrf.tile([128, 1], BF, tag="ecol", bufs=1)
                nc.tensor.transpose(pcol[:], esc[:], id1b[:])
                nc.scalar.copy(cur[:], pcol[:])
                lg = tmppool.tile([1, BL], F32, tag="elg")
                nc.scalar.activation(lg[:], mx[:], AF.Ln)
                nc.vector.tensor_tensor(out=c_row[:], in0=c_row[:], in1=lg[:], op=OP.add)

            if n_main > 0:
                with tc.For_i(0, n_main, 1) as civ:
                    cur, nxt = e_a, e_b
                    for u in range(crf_k):
                        crf_step(cur, nxt, ds(civ * crf_k + 1 + u, 1))
                        cur, nxt = nxt, cur
                    rescale(e_a)
            cur, nxt = e_a, e_b
            for u in range(tail):
                crf_step(cur, nxt, ds(n_main * crf_k + 1 + u, 1))
                cur, nxt = nxt, cur
            e_fin = cur

            # logZ
            v = tmppool.tile([128, 1], F32, tag="vfin")
            nc.vector.tensor_tensor(out=v[:], in0=e_fin[:], in1=expend[:], op=OP.mult)
            pv = pscrf.tile([1, 128], F32, tag="vrow", bufs=1)
            nc.tensor.transpose(pv[:], v[:], id128f[:])
            vs = tmppool.tile([1, BL], F32, tag="vseg")
            nc.vector.tensor_reduce(out=vs[:], in_=pv.rearrange("o (b j) -> o b j", b=BL),
                                    axis=mybir.AxisListType.X, op=OP.add)
            lz = tmppool.tile([1, BL], F32, tag="lz")
            nc.scalar.activation(lz[:], vs[:], AF.Ln)
            nc.vector.tensor_tensor(out=lz[:], in0=lz[:], in1=c_row[:], op=OP.add)
            lzt = tmppool.tile([1, 1], F32, tag="lzt")
            nc.vector.tensor_reduce(out=lzt[:], in_=lz[:],
                                    axis=mybir.AxisListType.X, op=OP.add)

            pscrf.release()
            res = tmppool.tile([1, 1], F32, tag="res")
            nc.vector.tensor_tensor(out=res[:], in0=lzt[:], in1=score_tot[:],
                                    op=OP.subtract)
            resout = C([1, 1], F32, "resout")
            nc.scalar.mul(resout[:], res[:], 1.0 / B)
            nc.sync.dma_start(out.ap(), resout[:])

            if debug_outs:
                nc.sync.dma_start(dbg_em.ap(), em_row[:])
                dbgh = bigpool.tile([128, 4 * SLOT * BL], F32, tag="dbgh", name="dbgh")
                nc.vector.tensor_copy(dbgh[:], h_all[LAYERS - 1][:])
                nc.sync.dma_start(dbg_h.ap(), dbgh[:])

    nc.compile()
    return nc


# ---------------- host-side prep ----------------

def _prep_shared(inputs):
    emb = np.asarray(inputs["embedding"], np.float32).astype(ml_dtypes.bfloat16)
    lp = inputs["lstm_params"]
    shared = {"emb": emb}
    for l in range(LAYERS):
        for d in range(2):
            p = lp[l][d]
            wih_p = _perm_rows(np.asarray(p["Wih"], np.float32))
            whh_p = _perm_rows(np.asarray(p["Whh"], np.float32))
            b_p = _perm_rows((np.asarray(p["bih"], np.float32)
                              + np.asarray(p["bhh"], np.float32))[:, None])[:, 0]
            wihT = wih_p.T  # [in_dim, 4H]
            n_k = wihT.shape[0] // 128
            shared[f"wih{l}{d}"] = np.ascontiguousarray(
                wihT.reshape(n_k, 128, 8, 128).transpose(1, 0, 2, 3).reshape(
                    128, n_k * 8 * 128)).astype(ml_dtypes.bfloat16)
            whhT = whh_p.T  # [H, 4H]
            shared[f"whh{l}{d}"] = np.ascontiguousarray(
                whhT.reshape(2, 128, 8, 128).transpose(1, 2, 0, 3).reshape(
                    128, 16 * 128)).astype(ml_dtypes.bfloat16)
            shared[f"bias{l}{d}"] = np.ascontiguousarray(
                b_p.reshape(8, 128).T).astype(np.float32)
    shared["woutT"] = np.ascontiguousarray(
        np.asarray(inputs["W_out"], np.float32).T.reshape(4, 128, TAGS)
        .transpose(1, 0, 2).reshape(128, 4 * TAGS)).astype(ml_dtypes.bfloat16)
    shared["bout"] = np.asarray(inputs["b_out"], np.float32)[None, :].astype(
        ml_dtypes.bfloat16)
    trans = np.asarray(inputs["transitions"], np.float32)
    blockraw = np.full((128, 128), NEG, np.float32)
    for b in range(BL):
        blockraw[32 * b:32 * b + 32, 32 * b:32 * b + 32] = trans
    shared["blockraw"] = blockraw
    start = np.asarray(inputs["start_transitions"], np.float32)
    end = np.asarray(inputs["end_transitions"], np.float32)
    shared["startrep"] = np.tile(start, BL)[:, None].astype(np.float32)
    shared["endrep"] = np.tile(end, BL)[:, None].astype(np.float32)
    p4 = np.zeros((128, BL), np.float32)
    p4[np.arange(128), np.arange(128) % BL] = 1.0
    shared["p4"] = p4
    shared["tables"] = np.concatenate(
        [trans, start[:, None], end[:, None]], axis=1).astype(np.float32)
    return shared


def _prep_core(x_c, tags_c, S):
    NTOK = S * BL
    NT = NTOK // 128
    mask = (x_c != 0).astype(np.float32)          # [BL, S]
    xperm = np.ascontiguousarray(x_c.T.reshape(NTOK, 1)).astype(np.int32)
    tagsT = tags_c.T                              # [S, BL]
    maskT = mask.T

    emone = np.zeros((NTOK, TAGS), np.float32)
    tok = np.arange(NTOK)
    emone[tok, tagsT.reshape(-1)] = maskT.reshape(-1)
    emone_t = np.ascontiguousarray(
        emone.reshape(NT, 128, TAGS).transpose(1, 0, 2).reshape(128, NT * TAGS))

    pfc = np.zeros((TAGS, NTOK + 128), np.float32)
    ptc = np.zeros((NTOK + 128, 34), np.float32)
    for t in range(S):
        for b in range(BL):
            c = t * BL + b
            if t == 0:
                pfc[tagsT[0, b], c] = 1.0
                ptc[c, 32] = 1.0
            else:
                pfc[tagsT[t - 1, b], c] = maskT[t, b]
                ptc[c, tagsT[t, b]] = 1.0
    for b in range(BL):
        c = NTOK + b
        pfc[tagsT[S - 1, b], c] = maskT[S - 1, b]
        ptc[c, 33] = 1.0
    pt_t = np.ascontiguousarray(
        ptc.reshape(NT + 1, 128, 34).transpose(1, 0, 2).reshape(128, (NT + 1) * 34))
    return {"xidx": xperm, "emone": emone_t, "pf": np.ascontiguousarray(pfc),
            "pt": pt_t}


_NC_CACHE = {}


def _get_nc(S=S_FULL):
    if S not in _NC_CACHE:
        _NC_CACHE[S] = build_nc(S=S)
    return _NC_CACHE[S]


def make_in_maps(inputs):
    x = np.asarray(inputs["x"])
    tags = np.asarray(inputs["tags"])
    S = x.shape[1]
    shared = _prep_shared(inputs)
    in_maps = []
    for c in range(NCORES):
        bl = slice(c * BL, (c + 1) * BL)
        m = dict(shared)
        m.update(_prep_core(x[bl], tags[bl], S))
        in_maps.append(m)
    return in_maps


def kernel(**inputs) -> np.ndarray:
    x = np.asarray(inputs["x"])
    S = x.shape[1]
    nc = _get_nc(S)
    in_maps = make_in_maps(inputs)
    res = run_bass_kernel_spmd(nc, in_maps, core_ids=list(range(NCORES)))
    total = np.float32(0.0)
    for c in range(NCORES):
        total += np.float32(res.results[c]["out"][0, 0])
    return np.float32(total)
